# revision 18
# baseline (speedup 1.0000x reference)
"""Trainium2 Bass kernel for nn_NMSquaredGaussianMixture.

Math: output = -(log(sum_n g_n^2) - log z) / N
  g_n = sum_k c_k * exp(E_k(x_n)),  c_k = w_k / sqrt((2pi)^2 det S_k)
  E_k(x) = -0.5 (x-mu_k)^T S_k^{-1} (x-mu_k)
  z     = sum_ij w_i w_j N(mu_i - mu_j; 0, S_i + S_j)   (tiny, host-side)

Device pipeline (per core, data-parallel over samples):
  mm1:  E = W1 @ F  in bf16 (1 cyc/row vs fp32r's fp32_mode=HIGH 2 cyc/row).
        F = 8 feature slots [y0^2, y0*y1, y1^2, y0, y0, y1, y1, y0^2] in a
        re-centered basis; the duplicated slots carry hi/lo bf16 splits of
        the linear (and one quadratic) coefficients, and the constant term
        (logc - 0.5 mPm) rides the fp32 activation bias instead of a bf16
        slot -- together this pushes the bf16-weight rounding bias on the
        final scalar from ~4e-3 down to ~1e-4 (measured offline).
  exp:  whole supertiles greedily load-balanced between ACT (two biased
        [128, 512] exp ops -> fp8e4, exact) and DVE (one [128, 2, 512]
        scalar_tensor_tensor fp8 Schraudolph: u8 bits ~= E*8/ln2 + B1[k,h]
        from a precomputed bias matrix; negative saturates to +0.0).
  mm2:  ONE fp8 DoubleRow matmul per supertile (lhsT [128, 2, 128] pairs
        the half0/half1 sign subtiles, 0.5 cyc/row): supertile st
        accumulates into the 16-row stripe 16*(st%8) of a [128, 512] PSUM
        tile; after 8 supertiles one ACT Square+accum_out row-reduces it
        into acc[:, block]. Output per core is acc [128, 4] f32 (2KB) --
        no wide writeback, no host-side 2M-element postprocess.

Measured on HW: supertile period ~663ns at 2.4GHz (PE roofline: 216+216
mm1 + ~230 DR mm2), exec ~45.5us/core of which ~10.5us is the walrus
postamble semaphore sweep and ~5us launch DMA/ramp -- see the memory notes
for the full cost model.

DMA: rhs chunk 0 is issued before the weights so the first matmul's data
is in flight during the fixed ~7us program preamble; warm-up matmuls run
on memset tiles (no DMA dependency) to heat the PE HAM clock-gate while
the first chunks land.
"""

import numpy as np

import concourse.bass as bass
import concourse.mybir as mybir
import concourse.tile as tile
from concourse import bacc
from concourse.bass_utils import run_bass_kernel_spmd

N_SAMPLES = 2_000_000
N_CORES = 8
NC_SAMP = N_SAMPLES // N_CORES  # 250_000
K = 16  # clusters
NF = 8  # feature slots
G = 16  # sample groups (one per matmul output column block)
FD = 512  # moving free dim (one PSUM bank of fp32)
SUPER = G * FD  # samples per super-tile = 8192
NST = 32  # geometry super-tiles/core (8-ST g-blocks)
NST_EMIT = 31  # ST31 is pure padding: skip it, its g stripe stays zero
NPAD = NST * SUPER  # 262144 padded samples per core
NBLK = NST // 8  # g-square blocks
PAD_U = 1.0e6  # pad feature: huge y0^2 --> E << 0 --> dens = 0

TWO_PI = 2.0 * np.pi
# bf16-Schraudolph exp constants: bits(bf16 exp(v)) ~= v*128/ln2 + (127*128-C2)
# C2 = 8.0 calibrated end-to-end (cancels the piecewise-linear bias).
SCHRAUD_A1 = float(8.0 / np.log(2.0))
SCHRAUD_B1 = float(7.0 * 8.0 - 0.0)
# pipeline tuning knobs
RHS_CHUNKS = [2, 2, 4, 6, 6, 6, 5]
RHS_BUFS = 4
DENS_BUFS = 5
E_BUFS = 3
G_BUFS = 2
PIPE_D = 3  # supertile lag between mm1 emission and mm2 consumption
# exp engine per half-tile unit (u = st*2 + half), cycled: A=ACT exp,
# D=DVE fp8 Schraudolph.
NWARM = 4
WARM_FD = 512

_CACHE = {}


def _bf16_np():
    import ml_dtypes

    return ml_dtypes.bfloat16


def _cluster_params(means, chols, weights):
    """Monomial coefficients A [K,6] (f64) of E_k in a re-centered basis
    (A columns: y0^2, y0*y1, y1^2, y0, y1, const incl logc), signs [K],
    center ctr [2]."""
    means = np.asarray(means, np.float64)
    chols = np.asarray(chols, np.float64)
    weights = np.asarray(weights, np.float64)
    L = np.tril(chols)
    S = L @ np.swapaxes(L, 1, 2)
    P = np.linalg.inv(S)
    detS = np.linalg.det(S)
    c = weights / np.sqrt(TWO_PI**2 * detS)
    signs = np.where(c >= 0, 1.0, -1.0)
    logc = np.log(np.abs(c))
    pw = np.abs(P).sum(axis=(1, 2))
    ctr = (means * pw[:, None]).sum(0) / pw.sum()
    m = means - ctr[None, :]
    Pm = np.einsum("kij,kj->ki", P, m)
    mPm = np.einsum("ki,ki->k", m, Pm)
    A = np.stack(
        [
            -0.5 * P[:, 0, 0],
            -P[:, 0, 1],
            -0.5 * P[:, 1, 1],
            Pm[:, 0],
            Pm[:, 1],
            -0.5 * mPm + logc,
        ],
        axis=1,
    )
    return A, signs, ctr


def _z_term(means, chols, weights):
    means = np.asarray(means, np.float64)
    chols = np.asarray(chols, np.float64)
    weights = np.asarray(weights, np.float64)
    L = np.tril(chols)
    S = L @ np.swapaxes(L, 1, 2)
    Ssum = S[:, None] + S[None, :]
    mdiff = means[:, None, :] - means[None, :, :]
    m2 = np.einsum("abi,abij,abj->ab", mdiff, np.linalg.inv(Ssum), mdiff)
    Zij = np.exp(-0.5 * m2) / np.sqrt(TWO_PI**2 * np.linalg.det(Ssum))
    return float(np.einsum("i,j,ij->", weights, weights, Zij))


def _build_rhs(X, ctr):
    """X [2M,2] f32 -> per-core rhs [N_CORES, 128, NST*FD] bf16, where
    rhs[c, s*G+g, st*FD + t] = feat_s of sample n = c*NC_SAMP + st*SUPER
    + g*FD + t (pad samples give dens == 0). Slots s: [q0 q01 q1 x0 x0 x1
    x1 q0]."""
    bf16 = _bf16_np()
    X = np.asarray(X, np.float32)
    feats = np.zeros((N_CORES, NPAD, NF), np.float32)
    x0 = (X[:, 0] - np.float32(ctr[0])).reshape(N_CORES, NC_SAMP)
    x1 = (X[:, 1] - np.float32(ctr[1])).reshape(N_CORES, NC_SAMP)
    q0 = x0 * x0
    feats[:, :NC_SAMP, 0] = q0
    feats[:, :NC_SAMP, 1] = x0 * x1
    feats[:, :NC_SAMP, 2] = x1 * x1
    feats[:, :NC_SAMP, 3] = x0
    feats[:, :NC_SAMP, 4] = x0
    feats[:, :NC_SAMP, 5] = x1
    feats[:, :NC_SAMP, 6] = x1
    feats[:, :NC_SAMP, 7] = q0
    feats[:, NC_SAMP:, 0] = PAD_U
    feats[:, NC_SAMP:, 7] = PAD_U
    # [C, NST, G, FD, NF] -> [C, NF, G, NST, FD] -> [C, 128, NST*FD]
    r = feats.reshape(N_CORES, NST, G, FD, NF).transpose(0, 4, 2, 1, 3)
    r = np.ascontiguousarray(r).reshape(N_CORES, NF * G, NST * FD)
    return r[:, :, : NST_EMIT * FD].astype(bf16)


def _build_weights(A, signs):
    """w1 [128, 256] bf16 block coefficient mats (cluster halves);
    cm [128, 32] bf16 sign-combine mats; cv [128, 4] f32 per-partition
    constants (exp bias h0/h1, Schraudolph B2 h0/h1).

    Out partition m = kl*G + g (kl = cluster within half). Contraction
    partition p = s*G + g. Slot weights: [hi(A0), A1, A2, hi(A3), lo(A3),
    hi(A4), lo(A4), lo(A0)]; const A5 rides the activation bias."""
    bf16 = _bf16_np()

    def hi_lo(v):
        h = v.astype(bf16).astype(np.float64)
        l = (v - h).astype(bf16).astype(np.float64)
        return h, l

    a0h, a0l = hi_lo(A[:, 0])
    a3h, a3l = hi_lo(A[:, 3])
    a4h, a4l = hi_lo(A[:, 4])
    slotw = np.stack(
        [a0h, A[:, 1], A[:, 2], a3h, a3l, a4h, a4l, a0l], axis=1
    )  # [K, 8]
    const = A[:, 5]

    import ml_dtypes

    f8 = ml_dtypes.float8_e4m3fn
    logcmax = float(const.max())  # E + const <= const <= logcmax => dens <= 1
    w1 = np.zeros((NF * G, 2 * 128), np.float32)
    # cm: DoubleRow lhsT [128, 2*8, 128]: for variant v = st%8 the matmul
    # uses subtile pair (2v, 2v+1) = (half0, half1) signs, nonzero only in
    # out rows v*16..v*16+16 -- eight super-tiles accumulate into disjoint
    # 16-row stripes of one full [128, FD] PSUM tile (DoubleRow dst
    # partition base must be 0, so the out always spans all 128 rows).
    cm = np.zeros((128, 16, 128), np.float32)
    cv = np.zeros((128, 4), np.float32)
    for half in (0, 1):
        for kl in range(8):
            k = half * 8 + kl
            for g in range(G):
                m = kl * G + g
                for s in range(NF):
                    w1[s * G + g, half * 128 + m] = slotw[k, s]
                for v in range(8):
                    cm[m, v * 2 + half, v * 16 + g] = signs[k]
                cv[m, half] = const[k] - logcmax
                cv[m, 2 + half] = (const[k] - logcmax) * SCHRAUD_A1 + SCHRAUD_B1
    return w1.astype(bf16), cm.reshape(128, 16 * 128).astype(f8), cv, logcmax


def _build_bass():
    nc = bacc.Bacc("TRN2", target_bir_lowering=False, debug=False)
    f32 = mybir.dt.float32
    bf16 = mybir.dt.bfloat16
    f8 = mybir.dt.float8e4
    u8 = mybir.dt.uint8
    rhs_d = nc.dram_tensor("rhs", [NF * G, NST_EMIT * FD], bf16, kind="ExternalInput")
    w1_d = nc.dram_tensor("w1", [NF * G, 2 * 128], bf16, kind="ExternalInput")
    cm_d = nc.dram_tensor("cm", [128, 16 * 128], f8, kind="ExternalInput")
    cv_d = nc.dram_tensor("cv", [128, 4], f32, kind="ExternalInput")
    acc_d = nc.dram_tensor("acc", [128, NBLK], f32, kind="ExternalOutput")

    assert sum(RHS_CHUNKS) == NST_EMIT

    with tile.TileContext(nc) as tc:
        with (
            tc.tile_pool(name="const", bufs=1) as cpool,
            tc.tile_pool(name="rhs", bufs=RHS_BUFS) as rpool,
            tc.tile_pool(name="dens", bufs=DENS_BUFS) as dpool,
            tc.tile_pool(name="pe", bufs=E_BUFS, space="PSUM") as epool,
            tc.tile_pool(name="pg", bufs=G_BUFS, space="PSUM") as gpool,
        ):
            w1 = cpool.tile([NF * G, 2 * 128], bf16)
            cm = cpool.tile([128, 16, 128], f8)
            cv = cpool.tile([128, 4], f32)
            acc = cpool.tile([128, NBLK], f32)
            scr = cpool.tile([128, FD], bf16)  # square dummy out
            wsb = cpool.tile([128, FD], bf16)  # warm-up operand
            b2m = cpool.tile([128, 2, FD], f32)  # Schraudolph bias matrix
            zro = cpool.tile([128, FD], f32)

            # rhs chunk 0 first: its transfer overlaps the remaining issue
            # chain and the program preamble.
            # Issue order = need order: w1 (first mm1), rhs chunk 0,
            # cv (first exp bias), remaining chunks, cm (first mm2 comes
            # PIPE_D supertiles in).
            nc.sync.dma_start(w1[:], w1_d[:])
            rhs_views = {}
            chunk_tiles = []
            lo = 0
            for ci, sz in enumerate(RHS_CHUNKS):
                hi = lo + sz
                rt = rpool.tile([NF * G, max(RHS_CHUNKS) * FD], bf16, tag="rhs")
                nc.sync.dma_start(rt[:, : sz * FD], rhs_d[:, lo * FD : hi * FD])
                for st in range(lo, hi):
                    rhs_views[st] = rt[:, (st - lo) * FD : (st - lo + 1) * FD]
                lo = hi
                chunk_tiles.append(rt)
                if ci == 0:
                    nc.sync.dma_start(cv[:], cv_d[:])
                elif ci == 1:
                    nc.sync.dma_start(cm[:, :, :], cm_d[:])

            # b2m[:, h, :] = cv[:, 2+h] broadcast along free: lets one DVE
            # scalar_tensor_tensor cover a whole supertile (both halves) in
            # a single op despite the per-half Schraudolph bias. Built on
            # DVE during the DMA-wait window (GpSimd elementwise measures
            # ~7.5us per [128, 512] op -- useless for compute).
            nc.vector.memset(zro[:], 0.0)
            for h in (0, 1):
                nc.vector.tensor_scalar(
                    b2m[:, h, :],
                    zro[:],
                    cv[:, 2 + h : 3 + h],
                    None,
                    op0=mybir.AluOpType.add,
                )
            # PE warm-up on memset tiles (no DMA dependency): keeps the PE
            # busy from program start so the HAM clock-gate window fills
            # exactly while the first rhs chunk lands; real matmuls then
            # run at 2.4 GHz almost immediately.
            nc.vector.memset(wsb[:], 0.0)
            warm = gpool.tile([128, FD], f32, tag="g", name="warm")
            for _ in range(NWARM):
                nc.tensor.matmul(
                    warm[0:128, 0:WARM_FD],
                    wsb[:, 0:128],
                    wsb[:, 0:WARM_FD],
                    start=True,
                    stop=True,
                )

            dens_ring = [None] * NST
            g_ring = [None, None]

            # Greedy exp-engine load balance (ns, approximate measured
            # costs): ACT does whole-ST exp as two biased [128, FD] ops;
            # DVE does whole-ST exp as one [128, 2, FD]
            # scalar_tensor_tensor against the b2m bias matrix. Squares
            # land on ACT (one per 8-ST block).
            ACT_ST, DVE_ST, SQ = 1480.0, 1280.0, 1000.0
            eng_load = {"A": 0.0, "D": 0.0}

            def emit_front(st):
                rhs = rhs_views[st]
                e = epool.tile([128, 2, FD], f32, tag="e")
                d = dpool.tile([128, 2, FD], f8, tag="dens")
                nc.tensor.matmul(
                    e[:, 0, :], w1[:, 0:128], rhs, start=True, stop=True
                )
                nc.tensor.matmul(
                    e[:, 1, :], w1[:, 128:256], rhs, start=True, stop=True
                )
                if eng_load["A"] + ACT_ST <= eng_load["D"] + DVE_ST:
                    eng_load["A"] += ACT_ST
                    for h in (0, 1):
                        nc.scalar.activation(
                            d[:, h, :],
                            e[:, h, :],
                            mybir.ActivationFunctionType.Exp,
                            bias=cv[:, h : h + 1],
                        )
                else:
                    eng_load["D"] += DVE_ST
                    nc.vector.scalar_tensor_tensor(
                        d[:, :, :].bitcast(u8),
                        e[:, :, :],
                        SCHRAUD_A1,
                        b2m[:, :, :],
                        op0=mybir.AluOpType.mult,
                        op1=mybir.AluOpType.add,
                    )
                dens_ring[st] = d

            def emit_back(st):
                d = dens_ring[st]
                dens_ring[st] = None
                v = st % 8
                if v == 0:
                    g_ring[(st // 8) % 2] = gpool.tile(
                        [128, FD], f32, tag="g", name=f"g128_{st // 8}"
                    )
                g128 = g_ring[(st // 8) % 2]
                nc.tensor.matmul(
                    g128[:, :],
                    cm[:, 2 * v : 2 * v + 2, :],
                    d[:, :, :],
                    start=(v == 0),
                    stop=(v == 7 or st == NST_EMIT - 1),
                    perf_mode=mybir.MatmulPerfMode.DoubleRow,
                    skip_group_check=True,
                )
                if v == 7 or st == NST_EMIT - 1:
                    blk = st // 8
                    eng_load["A"] += SQ
                    nc.scalar.activation(
                        scr[:],
                        g128[:],
                        mybir.ActivationFunctionType.Square,
                        accum_out=acc[:, blk : blk + 1],
                    )
                    if blk == NBLK - 2:
                        nc.sync.dma_start(
                            acc_d[:, : NBLK - 1], acc[:, : NBLK - 1]
                        )

            D = PIPE_D
            for t in range(NST_EMIT + D):
                if t < NST_EMIT:
                    emit_front(t)
                if t >= D:
                    emit_back(t - D)

            nc.sync.dma_start(
                acc_d[:, NBLK - 1 : NBLK], acc[:, NBLK - 1 : NBLK]
            )

    nc.compile()
    return nc


def _get_bass():
    if "nc" not in _CACHE:
        _CACHE["nc"] = _build_bass()
    return _CACHE["nc"]


def kernel(X, means, chols, weights, it=None, **_unused):
    X = np.ascontiguousarray(np.asarray(X, np.float32))
    assert X.shape == (N_SAMPLES, 2), X.shape

    A, signs, ctr = _cluster_params(means, chols, weights)
    z = _z_term(means, chols, weights)
    w1, cm, cv, logcmax = _build_weights(A, signs)
    rhs = _build_rhs(X, ctr)

    nc = _get_bass()
    in_maps = [
        {"rhs": rhs[c], "w1": w1, "cm": cm, "cv": cv} for c in range(N_CORES)
    ]
    res = run_bass_kernel_spmd(nc, in_maps, core_ids=list(range(N_CORES)))

    total = 0.0
    for r in res.results:
        total += float(r["acc"].astype(np.float64).sum())
    # dens were scaled by exp(-logcmax) on device; undo on the squared sum.
    out = -(np.log(total) + 2.0 * logcmax - np.log(z)) / N_SAMPLES
    return np.float32(out)


if __name__ == "__main__":
    rng = np.random.default_rng(0)
    X = rng.standard_normal((N_SAMPLES, 2), dtype=np.float32)
    scale = 2.0 * (1.0 + rng.standard_normal((K, 1, 1), dtype=np.float32))
    chols = scale * np.ones((2, 2), np.float32)[None] + 0.5 * np.eye(2, dtype=np.float32)[None]
    means = rng.standard_normal((K, 2), dtype=np.float32)
    weights = rng.standard_normal(K, dtype=np.float32)
    print(kernel(X, means, chols, weights, 1))


# revision 19
# speedup vs baseline: 1.0312x; 1.0312x over previous
"""Trainium2 Bass kernel for nn_NMSquaredGaussianMixture.

Math: output = -(log(sum_n g_n^2) - log z) / N
  g_n = sum_k c_k * exp(E_k(x_n)),  c_k = w_k / sqrt((2pi)^2 det S_k)
  E_k(x) = -0.5 (x-mu_k)^T S_k^{-1} (x-mu_k)
  z     = sum_ij w_i w_j N(mu_i - mu_j; 0, S_i + S_j)   (tiny, host-side)

Device pipeline (per core, data-parallel over samples):
  mm1:  E = W1 @ F  in bf16 (1 cyc/row vs fp32r's fp32_mode=HIGH 2 cyc/row).
        F = 8 feature slots [y0^2, y0*y1, y1^2, y0, y0, y1, y1, y0^2] in a
        re-centered basis; the duplicated slots carry hi/lo bf16 splits of
        the linear (and one quadratic) coefficients, and the constant term
        (logc - 0.5 mPm) rides the fp32 activation bias instead of a bf16
        slot -- together this pushes the bf16-weight rounding bias on the
        final scalar from ~4e-3 down to ~1e-4 (measured offline).
  exp:  whole supertiles greedily load-balanced between ACT (two biased
        [128, 512] exp ops -> fp8e4, exact) and DVE (one [128, 2, 512]
        scalar_tensor_tensor fp8 Schraudolph: u8 bits ~= E*8/ln2 + B1[k,h]
        from a precomputed bias matrix; negative saturates to +0.0).
  mm2:  ONE fp8 DoubleRow matmul per supertile (lhsT [128, 2, 128] pairs
        the half0/half1 sign subtiles, 0.5 cyc/row): supertile st
        accumulates into the 16-row stripe 16*(st%8) of a [128, 512] PSUM
        tile; after 8 supertiles one ACT Square+accum_out row-reduces it
        into acc[:, block]. Output per core is acc [128, 4] f32 (2KB) --
        no wide writeback, no host-side 2M-element postprocess.

Measured on HW: supertile period ~663ns at 2.4GHz (PE roofline: 216+216
mm1 + ~230 DR mm2), exec ~45.5us/core of which ~10.5us is the walrus
postamble semaphore sweep and ~5us launch DMA/ramp -- see the memory notes
for the full cost model.

DMA: rhs chunk 0 is issued before the weights so the first matmul's data
is in flight during the fixed ~7us program preamble; warm-up matmuls run
on memset tiles (no DMA dependency) to heat the PE HAM clock-gate while
the first chunks land.
"""

import numpy as np

import concourse.bass as bass
import concourse.mybir as mybir
import concourse.tile as tile
from concourse import bacc
from concourse.bass_utils import run_bass_kernel_spmd

N_SAMPLES = 2_000_000
N_CORES = 8
NC_SAMP = N_SAMPLES // N_CORES  # 250_000
K = 16  # clusters
NF = 8  # feature slots
G = 16  # sample groups (one per matmul output column block)
FD = 512  # moving free dim (one PSUM bank of fp32)
SUPER = G * FD  # samples per super-tile = 8192
NST = 32  # geometry super-tiles/core (8-ST g-blocks)
NST_EMIT = 31  # ST31 is pure padding: skip it, its g stripe stays zero
NPAD = NST * SUPER  # 262144 padded samples per core
NBLK = NST // 8  # g-square blocks
PAD_U = 1.0e6  # pad feature: huge y0^2 --> E << 0 --> dens = 0

TWO_PI = 2.0 * np.pi
# bf16-Schraudolph exp constants: bits(bf16 exp(v)) ~= v*128/ln2 + (127*128-C2)
# C2 = 8.0 calibrated end-to-end (cancels the piecewise-linear bias).
SCHRAUD_A1 = float(8.0 / np.log(2.0))
SCHRAUD_B1 = float(7.0 * 8.0 - 0.0)
# pipeline tuning knobs
RHS_CHUNKS = [2, 2, 4, 6, 6, 6, 5]
RHS_BUFS = 4
DENS_BUFS = 5
E_BUFS = 3
G_BUFS = 2
PIPE_D = 3  # supertile lag between mm1 emission and mm2 consumption
# exp engine per half-tile unit (u = st*2 + half), cycled: A=ACT exp,
# D=DVE fp8 Schraudolph.
NWARM = 4
WARM_FD = 512

_CACHE = {}


def _bf16_np():
    import ml_dtypes

    return ml_dtypes.bfloat16


def _cluster_params(means, chols, weights):
    """Monomial coefficients A [K,6] (f64) of E_k in a re-centered basis
    (A columns: y0^2, y0*y1, y1^2, y0, y1, const incl logc), signs [K],
    center ctr [2]."""
    means = np.asarray(means, np.float64)
    chols = np.asarray(chols, np.float64)
    weights = np.asarray(weights, np.float64)
    L = np.tril(chols)
    S = L @ np.swapaxes(L, 1, 2)
    P = np.linalg.inv(S)
    detS = np.linalg.det(S)
    c = weights / np.sqrt(TWO_PI**2 * detS)
    signs = np.where(c >= 0, 1.0, -1.0)
    logc = np.log(np.abs(c))
    pw = np.abs(P).sum(axis=(1, 2))
    ctr = (means * pw[:, None]).sum(0) / pw.sum()
    m = means - ctr[None, :]
    Pm = np.einsum("kij,kj->ki", P, m)
    mPm = np.einsum("ki,ki->k", m, Pm)
    A = np.stack(
        [
            -0.5 * P[:, 0, 0],
            -P[:, 0, 1],
            -0.5 * P[:, 1, 1],
            Pm[:, 0],
            Pm[:, 1],
            -0.5 * mPm + logc,
        ],
        axis=1,
    )
    return A, signs, ctr


def _z_term(means, chols, weights):
    means = np.asarray(means, np.float64)
    chols = np.asarray(chols, np.float64)
    weights = np.asarray(weights, np.float64)
    L = np.tril(chols)
    S = L @ np.swapaxes(L, 1, 2)
    Ssum = S[:, None] + S[None, :]
    mdiff = means[:, None, :] - means[None, :, :]
    m2 = np.einsum("abi,abij,abj->ab", mdiff, np.linalg.inv(Ssum), mdiff)
    Zij = np.exp(-0.5 * m2) / np.sqrt(TWO_PI**2 * np.linalg.det(Ssum))
    return float(np.einsum("i,j,ij->", weights, weights, Zij))


def _build_rhs(X, ctr):
    """X [2M,2] f32 -> per-core rhs [N_CORES, 128, NST*FD] bf16, where
    rhs[c, s*G+g, st*FD + t] = feat_s of sample n = c*NC_SAMP + st*SUPER
    + g*FD + t (pad samples give dens == 0). Slots s: [q0 q01 q1 x0 x0 x1
    x1 q0]."""
    bf16 = _bf16_np()
    X = np.asarray(X, np.float32)
    feats = np.zeros((N_CORES, NPAD, NF), np.float32)
    x0 = (X[:, 0] - np.float32(ctr[0])).reshape(N_CORES, NC_SAMP)
    x1 = (X[:, 1] - np.float32(ctr[1])).reshape(N_CORES, NC_SAMP)
    q0 = x0 * x0
    feats[:, :NC_SAMP, 0] = q0
    feats[:, :NC_SAMP, 1] = x0 * x1
    feats[:, :NC_SAMP, 2] = x1 * x1
    feats[:, :NC_SAMP, 3] = x0
    feats[:, :NC_SAMP, 4] = x0
    feats[:, :NC_SAMP, 5] = x1
    feats[:, :NC_SAMP, 6] = x1
    feats[:, :NC_SAMP, 7] = q0
    feats[:, NC_SAMP:, 0] = PAD_U
    feats[:, NC_SAMP:, 7] = PAD_U
    # [C, NST, G, FD, NF] -> [C, NF, G, NST, FD] -> [C, 128, NST*FD]
    r = feats.reshape(N_CORES, NST, G, FD, NF).transpose(0, 4, 2, 1, 3)
    r = np.ascontiguousarray(r).reshape(N_CORES, NF * G, NST * FD)
    return r[:, :, : NST_EMIT * FD].astype(bf16)


def _build_weights(A, signs):
    """w1 [128, 256] bf16 block coefficient mats (cluster halves);
    cm [128, 32] bf16 sign-combine mats; cv [128, 4] f32 per-partition
    constants (exp bias h0/h1, Schraudolph B2 h0/h1).

    Out partition m = kl*G + g (kl = cluster within half). Contraction
    partition p = s*G + g. Slot weights: [hi(A0), A1, A2, hi(A3), lo(A3),
    hi(A4), lo(A4), lo(A0)]; const A5 rides the activation bias."""
    bf16 = _bf16_np()

    def hi_lo(v):
        h = v.astype(bf16).astype(np.float64)
        l = (v - h).astype(bf16).astype(np.float64)
        return h, l

    a0h, a0l = hi_lo(A[:, 0])
    a3h, a3l = hi_lo(A[:, 3])
    a4h, a4l = hi_lo(A[:, 4])
    slotw = np.stack(
        [a0h, A[:, 1], A[:, 2], a3h, a3l, a4h, a4l, a0l], axis=1
    )  # [K, 8]
    const = A[:, 5]

    import ml_dtypes

    f8 = ml_dtypes.float8_e4m3fn
    logcmax = float(const.max())  # E + const <= const <= logcmax => dens <= 1
    w1 = np.zeros((NF * G, 2 * 128), np.float32)
    # cm: DoubleRow lhsT [128, 2*8, 128]: for variant v = st%8 the matmul
    # uses subtile pair (2v, 2v+1) = (half0, half1) signs, nonzero only in
    # out rows v*16..v*16+16 -- eight super-tiles accumulate into disjoint
    # 16-row stripes of one full [128, FD] PSUM tile (DoubleRow dst
    # partition base must be 0, so the out always spans all 128 rows).
    cm = np.zeros((128, 16, 128), np.float32)
    cv = np.zeros((128, 4), np.float32)
    for half in (0, 1):
        for kl in range(8):
            k = half * 8 + kl
            for g in range(G):
                m = kl * G + g
                for s in range(NF):
                    w1[s * G + g, half * 128 + m] = slotw[k, s]
                for v in range(8):
                    cm[m, v * 2 + half, v * 16 + g] = signs[k]
                cv[m, half] = const[k] - logcmax
                cv[m, 2 + half] = (const[k] - logcmax) * SCHRAUD_A1 + SCHRAUD_B1
    return w1.astype(bf16), cm.reshape(128, 16 * 128).astype(f8), cv, logcmax


def _build_bass():
    nc = bacc.Bacc("TRN2", target_bir_lowering=False, debug=False)
    f32 = mybir.dt.float32
    bf16 = mybir.dt.bfloat16
    f8 = mybir.dt.float8e4
    u8 = mybir.dt.uint8
    rhs_d = nc.dram_tensor("rhs", [NF * G, NST_EMIT * FD], bf16, kind="ExternalInput")
    w1_d = nc.dram_tensor("w1", [NF * G, 2 * 128], bf16, kind="ExternalInput")
    cm_d = nc.dram_tensor("cm", [128, 16 * 128], f8, kind="ExternalInput")
    cv_d = nc.dram_tensor("cv", [128, 4], f32, kind="ExternalInput")
    acc_d = nc.dram_tensor("acc", [128, NBLK], f32, kind="ExternalOutput")

    assert sum(RHS_CHUNKS) == NST_EMIT

    with tile.TileContext(nc) as tc:
        with (
            tc.tile_pool(name="const", bufs=1) as cpool,
            tc.tile_pool(name="rhs", bufs=RHS_BUFS) as rpool,
            tc.tile_pool(name="dens", bufs=DENS_BUFS) as dpool,
            tc.tile_pool(name="pe", bufs=E_BUFS, space="PSUM") as epool,
            tc.tile_pool(name="pg", bufs=G_BUFS, space="PSUM") as gpool,
        ):
            w1 = cpool.tile([NF * G, 2 * 128], bf16)
            cm = cpool.tile([128, 16, 128], f8)
            cv = cpool.tile([128, 4], f32)
            acc = cpool.tile([128, NBLK], f32)
            scr = cpool.tile([128, FD], bf16)  # square dummy out
            wsb = cpool.tile([128, FD], bf16)  # warm-up operand
            b2m = cpool.tile([128, 2, FD], f32)  # Schraudolph bias matrix
            zro = cpool.tile([128, FD], f32)

            # rhs chunk 0 first: its transfer overlaps the remaining issue
            # chain and the program preamble.
            # Issue order = need order: w1 (first mm1), rhs chunk 0,
            # cv (first exp bias), remaining chunks, cm (first mm2 comes
            # PIPE_D supertiles in).
            nc.sync.dma_start(w1[:], w1_d[:])
            rhs_views = {}
            chunk_tiles = []
            lo = 0
            for ci, sz in enumerate(RHS_CHUNKS):
                hi = lo + sz
                rt = rpool.tile([NF * G, max(RHS_CHUNKS) * FD], bf16, tag="rhs")
                nc.sync.dma_start(rt[:, : sz * FD], rhs_d[:, lo * FD : hi * FD])
                for st in range(lo, hi):
                    rhs_views[st] = rt[:, (st - lo) * FD : (st - lo + 1) * FD]
                lo = hi
                chunk_tiles.append(rt)
                if ci == 0:
                    nc.sync.dma_start(cv[:], cv_d[:])
                elif ci == 1:
                    nc.sync.dma_start(cm[:, :, :], cm_d[:])

            # b2m[:, h, :] = cv[:, 2+h] broadcast along free: lets one DVE
            # scalar_tensor_tensor cover a whole supertile (both halves) in
            # a single op despite the per-half Schraudolph bias. Built on
            # DVE during the DMA-wait window (GpSimd elementwise measures
            # ~7.5us per [128, 512] op -- useless for compute).
            nc.vector.memset(zro[:], 0.0)
            for h in (0, 1):
                nc.vector.tensor_scalar(
                    b2m[:, h, :],
                    zro[:],
                    cv[:, 2 + h : 3 + h],
                    None,
                    op0=mybir.AluOpType.add,
                )
            # PE warm-up on memset tiles (no DMA dependency): keeps the PE
            # busy from program start so the HAM clock-gate window fills
            # exactly while the first rhs chunk lands; real matmuls then
            # run at 2.4 GHz almost immediately.
            nc.vector.memset(wsb[:], 0.0)
            warm = gpool.tile([128, FD], f32, tag="g", name="warm")
            for _ in range(NWARM):
                nc.tensor.matmul(
                    warm[0:128, 0:WARM_FD],
                    wsb[:, 0:128],
                    wsb[:, 0:WARM_FD],
                    start=True,
                    stop=True,
                )

            dens_ring = [None] * NST
            g_ring = [None, None]

            # Greedy exp-engine load balance (ns, approximate measured
            # costs): ACT does whole-ST exp as two biased [128, FD] ops;
            # DVE does whole-ST exp as one [128, 2, FD]
            # scalar_tensor_tensor against the b2m bias matrix. Squares
            # land on ACT (one per 8-ST block).
            ACT_ST, DVE_ST, SQ = 1374.0, 1280.0, 720.0
            eng_load = {"A": 0.0, "D": 0.0}

            def emit_front(st):
                rhs = rhs_views[st]
                e = epool.tile([128, 2, FD], f32, tag="e")
                d = dpool.tile([128, 2, FD], f8, tag="dens")
                nc.tensor.matmul(
                    e[:, 0, :], w1[:, 0:128], rhs, start=True, stop=True
                )
                nc.tensor.matmul(
                    e[:, 1, :], w1[:, 128:256], rhs, start=True, stop=True
                )
                if eng_load["A"] + ACT_ST <= eng_load["D"] + DVE_ST:
                    eng_load["A"] += ACT_ST
                    for h in (0, 1):
                        nc.scalar.activation(
                            d[:, h, :],
                            e[:, h, :],
                            mybir.ActivationFunctionType.Exp,
                            bias=cv[:, h : h + 1],
                        )
                else:
                    eng_load["D"] += DVE_ST
                    nc.vector.scalar_tensor_tensor(
                        d[:, :, :].bitcast(u8),
                        e[:, :, :],
                        SCHRAUD_A1,
                        b2m[:, :, :],
                        op0=mybir.AluOpType.mult,
                        op1=mybir.AluOpType.add,
                    )
                dens_ring[st] = d

            def emit_back(st):
                d = dens_ring[st]
                dens_ring[st] = None
                v = st % 8
                if v == 0:
                    g_ring[(st // 8) % 2] = gpool.tile(
                        [128, FD], f32, tag="g", name=f"g128_{st // 8}"
                    )
                g128 = g_ring[(st // 8) % 2]
                nc.tensor.matmul(
                    g128[:, :],
                    cm[:, 2 * v : 2 * v + 2, :],
                    d[:, :, :],
                    start=(v == 0),
                    stop=(v == 7 or st == NST_EMIT - 1),
                    perf_mode=mybir.MatmulPerfMode.DoubleRow,
                    skip_group_check=True,
                )
                if v == 7 or st == NST_EMIT - 1:
                    blk = st // 8
                    eng_load["A"] += SQ
                    nc.scalar.activation(
                        scr[:],
                        g128[:],
                        mybir.ActivationFunctionType.Square,
                        accum_out=acc[:, blk : blk + 1],
                    )
                    if blk == NBLK - 2:
                        nc.sync.dma_start(
                            acc_d[:, : NBLK - 1], acc[:, : NBLK - 1]
                        )

            D = PIPE_D
            for t in range(NST_EMIT + D):
                if t < NST_EMIT:
                    emit_front(t)
                if t >= D:
                    emit_back(t - D)

            nc.sync.dma_start(
                acc_d[:, NBLK - 1 : NBLK], acc[:, NBLK - 1 : NBLK]
            )

    nc.compile()
    return nc


def _get_bass():
    if "nc" not in _CACHE:
        _CACHE["nc"] = _build_bass()
    return _CACHE["nc"]


def kernel(X, means, chols, weights, it=None, **_unused):
    X = np.ascontiguousarray(np.asarray(X, np.float32))
    assert X.shape == (N_SAMPLES, 2), X.shape

    A, signs, ctr = _cluster_params(means, chols, weights)
    z = _z_term(means, chols, weights)
    w1, cm, cv, logcmax = _build_weights(A, signs)
    rhs = _build_rhs(X, ctr)

    nc = _get_bass()
    in_maps = [
        {"rhs": rhs[c], "w1": w1, "cm": cm, "cv": cv} for c in range(N_CORES)
    ]
    res = run_bass_kernel_spmd(nc, in_maps, core_ids=list(range(N_CORES)))

    total = 0.0
    for r in res.results:
        total += float(r["acc"].astype(np.float64).sum())
    # dens were scaled by exp(-logcmax) on device; undo on the squared sum.
    out = -(np.log(total) + 2.0 * logcmax - np.log(z)) / N_SAMPLES
    return np.float32(out)


if __name__ == "__main__":
    rng = np.random.default_rng(0)
    X = rng.standard_normal((N_SAMPLES, 2), dtype=np.float32)
    scale = 2.0 * (1.0 + rng.standard_normal((K, 1, 1), dtype=np.float32))
    chols = scale * np.ones((2, 2), np.float32)[None] + 0.5 * np.eye(2, dtype=np.float32)[None]
    means = rng.standard_normal((K, 2), dtype=np.float32)
    weights = rng.standard_normal(K, dtype=np.float32)
    print(kernel(X, means, chols, weights, 1))


# revision 20
# speedup vs baseline: 1.0399x; 1.0084x over previous
"""Trainium2 Bass kernel for nn_NMSquaredGaussianMixture.

Math: output = -(log(sum_n g_n^2) - log z) / N
  g_n = sum_k c_k * exp(E_k(x_n)),  c_k = w_k / sqrt((2pi)^2 det S_k)
  E_k(x) = -0.5 (x-mu_k)^T S_k^{-1} (x-mu_k)
  z     = sum_ij w_i w_j N(mu_i - mu_j; 0, S_i + S_j)   (tiny, host-side)

Device pipeline (per core, data-parallel over samples):
  mm1:  E = W1 @ F  in bf16 (1 cyc/row vs fp32r's fp32_mode=HIGH 2 cyc/row).
        F = 8 feature slots [y0^2, y0*y1, y1^2, y0, y0, y1, y1, y0^2] in a
        re-centered basis; the duplicated slots carry hi/lo bf16 splits of
        the linear (and one quadratic) coefficients, and the constant term
        (logc - 0.5 mPm) rides the fp32 activation bias instead of a bf16
        slot -- together this pushes the bf16-weight rounding bias on the
        final scalar from ~4e-3 down to ~1e-4 (measured offline).
  exp:  whole supertiles greedily load-balanced between ACT (two biased
        [128, 512] exp ops -> fp8e4, exact) and DVE (one [128, 2, 512]
        scalar_tensor_tensor fp8 Schraudolph: u8 bits ~= E*8/ln2 + B1[k,h]
        from a precomputed bias matrix; negative saturates to +0.0).
  mm2:  ONE fp8 DoubleRow matmul per supertile (lhsT [128, 2, 128] pairs
        the half0/half1 sign subtiles, 0.5 cyc/row): supertile st
        accumulates into the 16-row stripe 16*(st%8) of a [128, 512] PSUM
        tile; after 8 supertiles one ACT Square+accum_out row-reduces it
        into acc[:, block]. Output per core is acc [128, 4] f32 (2KB) --
        no wide writeback, no host-side 2M-element postprocess.

Measured on HW: supertile period ~663ns at 2.4GHz (PE roofline: 216+216
mm1 + ~230 DR mm2), exec ~45.5us/core of which ~10.5us is the walrus
postamble semaphore sweep and ~5us launch DMA/ramp -- see the memory notes
for the full cost model.

DMA: rhs chunk 0 is issued before the weights so the first matmul's data
is in flight during the fixed ~7us program preamble; warm-up matmuls run
on memset tiles (no DMA dependency) to heat the PE HAM clock-gate while
the first chunks land.
"""

import numpy as np

import concourse.bass as bass
import concourse.mybir as mybir
import concourse.tile as tile
from concourse import bacc
from concourse.bass_utils import run_bass_kernel_spmd

N_SAMPLES = 2_000_000
N_CORES = 8
NC_SAMP = N_SAMPLES // N_CORES  # 250_000
K = 16  # clusters
NF = 8  # feature slots
G = 16  # sample groups (one per matmul output column block)
FD = 512  # moving free dim (one PSUM bank of fp32)
SUPER = G * FD  # samples per super-tile = 8192
NST = 32  # geometry super-tiles/core (8-ST g-blocks)
NST_EMIT = 31  # ST31 is pure padding: skip it, its g stripe stays zero
NPAD = NST * SUPER  # 262144 padded samples per core
NBLK = NST // 8  # g-square blocks
PAD_U = 1.0e6  # pad feature: huge y0^2 --> E << 0 --> dens = 0

TWO_PI = 2.0 * np.pi
# bf16-Schraudolph exp constants: bits(bf16 exp(v)) ~= v*128/ln2 + (127*128-C2)
# C2 = 8.0 calibrated end-to-end (cancels the piecewise-linear bias).
SCHRAUD_A1 = float(8.0 / np.log(2.0))
SCHRAUD_B1 = float(7.0 * 8.0 - 0.0)
# pipeline tuning knobs
RHS_CHUNKS = [2, 2, 4, 6, 6, 6, 5]
RHS_BUFS = 4
DENS_BUFS = 5
E_BUFS = 3
G_BUFS = 2
PIPE_D = 3  # supertile lag between mm1 emission and mm2 consumption
# exp engine per half-tile unit (u = st*2 + half), cycled: A=ACT exp,
# D=DVE fp8 Schraudolph.
NWARM = 4
WARM_FD = 512

_CACHE = {}


def _bf16_np():
    import ml_dtypes

    return ml_dtypes.bfloat16


def _cluster_params(means, chols, weights):
    """Monomial coefficients A [K,6] (f64) of E_k in a re-centered basis
    (A columns: y0^2, y0*y1, y1^2, y0, y1, const incl logc), signs [K],
    center ctr [2]."""
    means = np.asarray(means, np.float64)
    chols = np.asarray(chols, np.float64)
    weights = np.asarray(weights, np.float64)
    L = np.tril(chols)
    S = L @ np.swapaxes(L, 1, 2)
    P = np.linalg.inv(S)
    detS = np.linalg.det(S)
    c = weights / np.sqrt(TWO_PI**2 * detS)
    signs = np.where(c >= 0, 1.0, -1.0)
    logc = np.log(np.abs(c))
    pw = np.abs(P).sum(axis=(1, 2))
    ctr = (means * pw[:, None]).sum(0) / pw.sum()
    m = means - ctr[None, :]
    Pm = np.einsum("kij,kj->ki", P, m)
    mPm = np.einsum("ki,ki->k", m, Pm)
    A = np.stack(
        [
            -0.5 * P[:, 0, 0],
            -P[:, 0, 1],
            -0.5 * P[:, 1, 1],
            Pm[:, 0],
            Pm[:, 1],
            -0.5 * mPm + logc,
        ],
        axis=1,
    )
    return A, signs, ctr


def _z_term(means, chols, weights):
    means = np.asarray(means, np.float64)
    chols = np.asarray(chols, np.float64)
    weights = np.asarray(weights, np.float64)
    L = np.tril(chols)
    S = L @ np.swapaxes(L, 1, 2)
    Ssum = S[:, None] + S[None, :]
    mdiff = means[:, None, :] - means[None, :, :]
    m2 = np.einsum("abi,abij,abj->ab", mdiff, np.linalg.inv(Ssum), mdiff)
    Zij = np.exp(-0.5 * m2) / np.sqrt(TWO_PI**2 * np.linalg.det(Ssum))
    return float(np.einsum("i,j,ij->", weights, weights, Zij))


def _build_rhs(X, ctr):
    """X [2M,2] f32 -> per-core rhs [N_CORES, 128, NST*FD] bf16, where
    rhs[c, s*G+g, st*FD + t] = feat_s of sample n = c*NC_SAMP + st*SUPER
    + g*FD + t (pad samples give dens == 0). Slots s: [q0 q01 q1 x0 x0 x1
    x1 q0]."""
    bf16 = _bf16_np()
    X = np.asarray(X, np.float32)
    feats = np.zeros((N_CORES, NPAD, NF), np.float32)
    x0 = (X[:, 0] - np.float32(ctr[0])).reshape(N_CORES, NC_SAMP)
    x1 = (X[:, 1] - np.float32(ctr[1])).reshape(N_CORES, NC_SAMP)
    q0 = x0 * x0
    feats[:, :NC_SAMP, 0] = q0
    feats[:, :NC_SAMP, 1] = x0 * x1
    feats[:, :NC_SAMP, 2] = x1 * x1
    feats[:, :NC_SAMP, 3] = x0
    feats[:, :NC_SAMP, 4] = x0
    feats[:, :NC_SAMP, 5] = x1
    feats[:, :NC_SAMP, 6] = x1
    feats[:, :NC_SAMP, 7] = q0
    feats[:, NC_SAMP:, 0] = PAD_U
    feats[:, NC_SAMP:, 7] = PAD_U
    # [C, NST, G, FD, NF] -> [C, NF, G, NST, FD] -> [C, 128, NST*FD]
    r = feats.reshape(N_CORES, NST, G, FD, NF).transpose(0, 4, 2, 1, 3)
    r = np.ascontiguousarray(r).reshape(N_CORES, NF * G, NST * FD)
    return r[:, :, : NST_EMIT * FD].astype(bf16)


def _build_weights(A, signs):
    """w1 [128, 256] bf16 block coefficient mats (cluster halves);
    cm [128, 32] bf16 sign-combine mats; cv [128, 4] f32 per-partition
    constants (exp bias h0/h1, Schraudolph B2 h0/h1).

    Out partition m = kl*G + g (kl = cluster within half). Contraction
    partition p = s*G + g. Slot weights: [hi(A0), A1, A2, hi(A3), lo(A3),
    hi(A4), lo(A4), lo(A0)]; const A5 rides the activation bias."""
    bf16 = _bf16_np()

    def hi_lo(v):
        h = v.astype(bf16).astype(np.float64)
        l = (v - h).astype(bf16).astype(np.float64)
        return h, l

    a0h, a0l = hi_lo(A[:, 0])
    a3h, a3l = hi_lo(A[:, 3])
    a4h, a4l = hi_lo(A[:, 4])
    slotw = np.stack(
        [a0h, A[:, 1], A[:, 2], a3h, a3l, a4h, a4l, a0l], axis=1
    )  # [K, 8]
    const = A[:, 5]

    import ml_dtypes

    f8 = ml_dtypes.float8_e4m3fn
    logcmax = float(const.max())  # E + const <= const <= logcmax => dens <= 1
    w1 = np.zeros((NF * G, 2 * 128), np.float32)
    # cm: DoubleRow lhsT [128, 2*8, 128]: for variant v = st%8 the matmul
    # uses subtile pair (2v, 2v+1) = (half0, half1) signs, nonzero only in
    # out rows v*16..v*16+16 -- eight super-tiles accumulate into disjoint
    # 16-row stripes of one full [128, FD] PSUM tile (DoubleRow dst
    # partition base must be 0, so the out always spans all 128 rows).
    cm = np.zeros((128, 16, 128), np.float32)
    cv = np.zeros((128, 4), np.float32)
    for half in (0, 1):
        for kl in range(8):
            k = half * 8 + kl
            for g in range(G):
                m = kl * G + g
                for s in range(NF):
                    w1[s * G + g, half * 128 + m] = slotw[k, s]
                for v in range(8):
                    cm[m, v * 2 + half, v * 16 + g] = signs[k]
                cv[m, half] = const[k] - logcmax
                cv[m, 2 + half] = (const[k] - logcmax) * SCHRAUD_A1 + SCHRAUD_B1
    wp = np.zeros((128, 528), np.uint8)
    wp[:, 0:512] = w1.astype(bf16).view(np.uint8)
    wp[:, 512:528] = cv.view(np.uint8)
    return wp, cm.reshape(128, 16 * 128).astype(f8), logcmax


def _build_bass():
    nc = bacc.Bacc("TRN2", target_bir_lowering=False, debug=False)
    f32 = mybir.dt.float32
    bf16 = mybir.dt.bfloat16
    f8 = mybir.dt.float8e4
    u8 = mybir.dt.uint8
    rhs_d = nc.dram_tensor("rhs", [NF * G, NST_EMIT * FD], bf16, kind="ExternalInput")
    wp_d = nc.dram_tensor("wp", [128, 528], u8, kind="ExternalInput")
    cm_d = nc.dram_tensor("cm", [128, 16 * 128], f8, kind="ExternalInput")
    acc_d = nc.dram_tensor("acc", [128, NBLK], f32, kind="ExternalOutput")

    assert sum(RHS_CHUNKS) == NST_EMIT

    with tile.TileContext(nc) as tc:
        with (
            tc.tile_pool(name="const", bufs=1) as cpool,
            tc.tile_pool(name="rhs", bufs=RHS_BUFS) as rpool,
            tc.tile_pool(name="dens", bufs=DENS_BUFS) as dpool,
            tc.tile_pool(name="pe", bufs=E_BUFS, space="PSUM") as epool,
            tc.tile_pool(name="pg", bufs=G_BUFS, space="PSUM") as gpool,
        ):
            wp = cpool.tile([128, 528], u8)
            cm = cpool.tile([128, 16, 128], f8)
            acc = cpool.tile([128, NBLK], f32)
            scr = cpool.tile([128, FD], bf16)  # square dummy out
            wsb = cpool.tile([128, FD], bf16)  # warm-up operand
            b2m = cpool.tile([128, 2, FD], f32)  # Schraudolph bias matrix
            zro = cpool.tile([128, FD], f32)

            # rhs chunk 0 first: its transfer overlaps the remaining issue
            # chain and the program preamble.
            # Issue order = need order: packed w1+cv (first mm1/exp), rhs
            # chunk 0, remaining chunks, cm (first mm2 comes PIPE_D
            # supertiles in). Packing w1+cv into one u8 tensor saves a
            # ~640ns serialized DMA issue on the head critical path.
            nc.sync.dma_start(wp[:], wp_d[:])

            def w1h(h):
                return wp[:, h * 256 : (h + 1) * 256].bitcast(bf16)

            def cvcol(i):
                return wp[:, 512 + 4 * i : 516 + 4 * i].bitcast(f32)
            rhs_views = {}
            chunk_tiles = []
            lo = 0
            for ci, sz in enumerate(RHS_CHUNKS):
                hi = lo + sz
                rt = rpool.tile([NF * G, max(RHS_CHUNKS) * FD], bf16, tag="rhs")
                nc.sync.dma_start(rt[:, : sz * FD], rhs_d[:, lo * FD : hi * FD])
                for st in range(lo, hi):
                    rhs_views[st] = rt[:, (st - lo) * FD : (st - lo + 1) * FD]
                lo = hi
                chunk_tiles.append(rt)
                if ci == 0:
                    nc.sync.dma_start(cm[:, :, :], cm_d[:])

            # b2m[:, h, :] = cv[:, 2+h] broadcast along free: lets one DVE
            # scalar_tensor_tensor cover a whole supertile (both halves) in
            # a single op despite the per-half Schraudolph bias. Built on
            # DVE during the DMA-wait window (GpSimd elementwise measures
            # ~7.5us per [128, 512] op -- useless for compute).
            nc.vector.memset(zro[:], 0.0)
            for h in (0, 1):
                nc.vector.tensor_scalar(
                    b2m[:, h, :],
                    zro[:],
                    cvcol(2 + h),
                    None,
                    op0=mybir.AluOpType.add,
                )
            # PE warm-up on memset tiles (no DMA dependency): keeps the PE
            # busy from program start so the HAM clock-gate window fills
            # exactly while the first rhs chunk lands; real matmuls then
            # run at 2.4 GHz almost immediately.
            nc.vector.memset(wsb[:], 0.0)
            warm = gpool.tile([128, FD], f32, tag="g", name="warm")
            for _ in range(NWARM):
                nc.tensor.matmul(
                    warm[0:128, 0:WARM_FD],
                    wsb[:, 0:128],
                    wsb[:, 0:WARM_FD],
                    start=True,
                    stop=True,
                )

            dens_ring = [None] * NST
            g_ring = [None, None]

            # Greedy exp-engine load balance (ns, approximate measured
            # costs): ACT does whole-ST exp as two biased [128, FD] ops;
            # DVE does whole-ST exp as one [128, 2, FD]
            # scalar_tensor_tensor against the b2m bias matrix. Squares
            # land on ACT (one per 8-ST block).
            ACT_ST, DVE_ST, SQ = 1374.0, 1280.0, 720.0
            eng_load = {"A": 0.0, "D": 0.0}

            def emit_front(st):
                rhs = rhs_views[st]
                e = epool.tile([128, 2, FD], f32, tag="e")
                d = dpool.tile([128, 2, FD], f8, tag="dens")
                nc.tensor.matmul(
                    e[:, 0, :], w1h(0), rhs, start=True, stop=True
                )
                nc.tensor.matmul(
                    e[:, 1, :], w1h(1), rhs, start=True, stop=True
                )
                if eng_load["A"] + ACT_ST <= eng_load["D"] + DVE_ST:
                    eng_load["A"] += ACT_ST
                    for h in (0, 1):
                        nc.scalar.activation(
                            d[:, h, :],
                            e[:, h, :],
                            mybir.ActivationFunctionType.Exp,
                            bias=cvcol(h),
                        )
                else:
                    eng_load["D"] += DVE_ST
                    nc.vector.scalar_tensor_tensor(
                        d[:, :, :].bitcast(u8),
                        e[:, :, :],
                        SCHRAUD_A1,
                        b2m[:, :, :],
                        op0=mybir.AluOpType.mult,
                        op1=mybir.AluOpType.add,
                    )
                dens_ring[st] = d

            def emit_back(st):
                d = dens_ring[st]
                dens_ring[st] = None
                v = st % 8
                if v == 0:
                    g_ring[(st // 8) % 2] = gpool.tile(
                        [128, FD], f32, tag="g", name=f"g128_{st // 8}"
                    )
                g128 = g_ring[(st // 8) % 2]
                nc.tensor.matmul(
                    g128[:, :],
                    cm[:, 2 * v : 2 * v + 2, :],
                    d[:, :, :],
                    start=(v == 0),
                    stop=(v == 7 or st == NST_EMIT - 1),
                    perf_mode=mybir.MatmulPerfMode.DoubleRow,
                    skip_group_check=True,
                )
                if v == 7 or st == NST_EMIT - 1:
                    blk = st // 8
                    eng_load["A"] += SQ
                    nc.scalar.activation(
                        scr[:],
                        g128[:],
                        mybir.ActivationFunctionType.Square,
                        accum_out=acc[:, blk : blk + 1],
                    )
                    if blk == NBLK - 2:
                        nc.sync.dma_start(
                            acc_d[:, : NBLK - 1], acc[:, : NBLK - 1]
                        )

            D = PIPE_D
            for t in range(NST_EMIT + D):
                if t < NST_EMIT:
                    emit_front(t)
                if t >= D:
                    emit_back(t - D)

            nc.sync.dma_start(
                acc_d[:, NBLK - 1 : NBLK], acc[:, NBLK - 1 : NBLK]
            )

    nc.compile()
    return nc


def _get_bass():
    if "nc" not in _CACHE:
        _CACHE["nc"] = _build_bass()
    return _CACHE["nc"]


def kernel(X, means, chols, weights, it=None, **_unused):
    X = np.ascontiguousarray(np.asarray(X, np.float32))
    assert X.shape == (N_SAMPLES, 2), X.shape

    A, signs, ctr = _cluster_params(means, chols, weights)
    z = _z_term(means, chols, weights)
    wp, cm, logcmax = _build_weights(A, signs)
    rhs = _build_rhs(X, ctr)

    nc = _get_bass()
    in_maps = [
        {"rhs": rhs[c], "wp": wp, "cm": cm} for c in range(N_CORES)
    ]
    res = run_bass_kernel_spmd(nc, in_maps, core_ids=list(range(N_CORES)))

    total = 0.0
    for r in res.results:
        total += float(r["acc"].astype(np.float64).sum())
    # dens were scaled by exp(-logcmax) on device; undo on the squared sum.
    out = -(np.log(total) + 2.0 * logcmax - np.log(z)) / N_SAMPLES
    return np.float32(out)


if __name__ == "__main__":
    rng = np.random.default_rng(0)
    X = rng.standard_normal((N_SAMPLES, 2), dtype=np.float32)
    scale = 2.0 * (1.0 + rng.standard_normal((K, 1, 1), dtype=np.float32))
    chols = scale * np.ones((2, 2), np.float32)[None] + 0.5 * np.eye(2, dtype=np.float32)[None]
    means = rng.standard_normal((K, 2), dtype=np.float32)
    weights = rng.standard_normal(K, dtype=np.float32)
    print(kernel(X, means, chols, weights, 1))


# revision 21
# speedup vs baseline: 1.0572x; 1.0167x over previous
"""Trainium2 Bass kernel for nn_NMSquaredGaussianMixture.

Math: output = -(log(sum_n g_n^2) - log z) / N
  g_n = sum_k c_k * exp(E_k(x_n)),  c_k = w_k / sqrt((2pi)^2 det S_k)
  E_k(x) = -0.5 (x-mu_k)^T S_k^{-1} (x-mu_k)
  z     = sum_ij w_i w_j N(mu_i - mu_j; 0, S_i + S_j)   (tiny, host-side)

Device pipeline (per core, data-parallel over samples):
  mm1:  E = W1 @ F  in bf16 (1 cyc/row vs fp32r's fp32_mode=HIGH 2 cyc/row).
        F = 8 feature slots [y0^2, y0*y1, y1^2, y0, y0, y1, y1, y0^2] in a
        re-centered basis; the duplicated slots carry hi/lo bf16 splits of
        the linear (and one quadratic) coefficients, and the constant term
        (logc - 0.5 mPm) rides the fp32 activation bias instead of a bf16
        slot -- together this pushes the bf16-weight rounding bias on the
        final scalar from ~4e-3 down to ~1e-4 (measured offline).
  exp:  whole supertiles greedily load-balanced between ACT (two biased
        [128, 512] exp ops -> fp8e4, exact) and DVE (one [128, 2, 512]
        scalar_tensor_tensor fp8 Schraudolph: u8 bits ~= E*8/ln2 + B1[k,h]
        from a precomputed bias matrix; negative saturates to +0.0).
  mm2:  ONE fp8 DoubleRow matmul per supertile (lhsT [128, 2, 128] pairs
        the half0/half1 sign subtiles, 0.5 cyc/row): supertile st
        accumulates into the 16-row stripe 16*(st%8) of a [128, 512] PSUM
        tile; after 8 supertiles one ACT Square+accum_out row-reduces it
        into acc[:, block]. Output per core is acc [128, 4] f32 (2KB) --
        no wide writeback, no host-side 2M-element postprocess.

Measured on HW: supertile period ~663ns at 2.4GHz (PE roofline: 216+216
mm1 + ~230 DR mm2), exec ~45.5us/core of which ~10.5us is the walrus
postamble semaphore sweep and ~5us launch DMA/ramp -- see the memory notes
for the full cost model.

DMA: rhs chunk 0 is issued before the weights so the first matmul's data
is in flight during the fixed ~7us program preamble; warm-up matmuls run
on memset tiles (no DMA dependency) to heat the PE HAM clock-gate while
the first chunks land.
"""

import numpy as np

import concourse.bass as bass
import concourse.mybir as mybir
import concourse.tile as tile
from concourse import bacc
from concourse.bass_utils import run_bass_kernel_spmd

N_SAMPLES = 2_000_000
N_CORES = 8
NC_SAMP = N_SAMPLES // N_CORES  # 250_000
K = 16  # clusters
NF = 8  # feature slots
G = 16  # sample groups (one per matmul output column block)
FD = 512  # moving free dim (one PSUM bank of fp32)
SUPER = G * FD  # samples per super-tile = 8192
NST = 32  # geometry super-tiles/core (8-ST g-blocks)
NST_EMIT = 31  # ST31 is pure padding: skip it, its g stripe stays zero
NPAD = NST * SUPER  # 262144 padded samples per core
NBLK = NST // 8  # g-square blocks
PAD_U = 1.0e6  # pad feature: huge y0^2 --> E << 0 --> dens = 0

TWO_PI = 2.0 * np.pi
# bf16-Schraudolph exp constants: bits(bf16 exp(v)) ~= v*128/ln2 + (127*128-C2)
# C2 = 8.0 calibrated end-to-end (cancels the piecewise-linear bias).
SCHRAUD_A1 = float(8.0 / np.log(2.0))
SCHRAUD_B1 = float(7.0 * 8.0 - 0.0)
# pipeline tuning knobs
RHS_CHUNKS = [1, 2, 4, 6, 6, 6, 6]
RHS_BUFS = 4
DENS_BUFS = 5
E_BUFS = 3
G_BUFS = 2
PIPE_D = 3  # supertile lag between mm1 emission and mm2 consumption
# exp engine per half-tile unit (u = st*2 + half), cycled: A=ACT exp,
# D=DVE fp8 Schraudolph.
NWARM = 3
WARM_FD = 512

_CACHE = {}


def _bf16_np():
    import ml_dtypes

    return ml_dtypes.bfloat16


def _cluster_params(means, chols, weights):
    """Monomial coefficients A [K,6] (f64) of E_k in a re-centered basis
    (A columns: y0^2, y0*y1, y1^2, y0, y1, const incl logc), signs [K],
    center ctr [2]."""
    means = np.asarray(means, np.float64)
    chols = np.asarray(chols, np.float64)
    weights = np.asarray(weights, np.float64)
    L = np.tril(chols)
    S = L @ np.swapaxes(L, 1, 2)
    P = np.linalg.inv(S)
    detS = np.linalg.det(S)
    c = weights / np.sqrt(TWO_PI**2 * detS)
    signs = np.where(c >= 0, 1.0, -1.0)
    logc = np.log(np.abs(c))
    pw = np.abs(P).sum(axis=(1, 2))
    ctr = (means * pw[:, None]).sum(0) / pw.sum()
    m = means - ctr[None, :]
    Pm = np.einsum("kij,kj->ki", P, m)
    mPm = np.einsum("ki,ki->k", m, Pm)
    A = np.stack(
        [
            -0.5 * P[:, 0, 0],
            -P[:, 0, 1],
            -0.5 * P[:, 1, 1],
            Pm[:, 0],
            Pm[:, 1],
            -0.5 * mPm + logc,
        ],
        axis=1,
    )
    return A, signs, ctr


def _z_term(means, chols, weights):
    means = np.asarray(means, np.float64)
    chols = np.asarray(chols, np.float64)
    weights = np.asarray(weights, np.float64)
    L = np.tril(chols)
    S = L @ np.swapaxes(L, 1, 2)
    Ssum = S[:, None] + S[None, :]
    mdiff = means[:, None, :] - means[None, :, :]
    m2 = np.einsum("abi,abij,abj->ab", mdiff, np.linalg.inv(Ssum), mdiff)
    Zij = np.exp(-0.5 * m2) / np.sqrt(TWO_PI**2 * np.linalg.det(Ssum))
    return float(np.einsum("i,j,ij->", weights, weights, Zij))


def _build_rhs(X, ctr):
    """X [2M,2] f32 -> per-core rhs [N_CORES, 128, NST*FD] bf16, where
    rhs[c, s*G+g, st*FD + t] = feat_s of sample n = c*NC_SAMP + st*SUPER
    + g*FD + t (pad samples give dens == 0). Slots s: [q0 q01 q1 x0 x0 x1
    x1 q0]."""
    bf16 = _bf16_np()
    X = np.asarray(X, np.float32)
    feats = np.zeros((N_CORES, NPAD, NF), np.float32)
    x0 = (X[:, 0] - np.float32(ctr[0])).reshape(N_CORES, NC_SAMP)
    x1 = (X[:, 1] - np.float32(ctr[1])).reshape(N_CORES, NC_SAMP)
    q0 = x0 * x0
    feats[:, :NC_SAMP, 0] = q0
    feats[:, :NC_SAMP, 1] = x0 * x1
    feats[:, :NC_SAMP, 2] = x1 * x1
    feats[:, :NC_SAMP, 3] = x0
    feats[:, :NC_SAMP, 4] = x0
    feats[:, :NC_SAMP, 5] = x1
    feats[:, :NC_SAMP, 6] = x1
    feats[:, :NC_SAMP, 7] = q0
    feats[:, NC_SAMP:, 0] = PAD_U
    feats[:, NC_SAMP:, 7] = PAD_U
    # [C, NST, G, FD, NF] -> [C, NF, G, NST, FD] -> [C, 128, NST*FD]
    r = feats.reshape(N_CORES, NST, G, FD, NF).transpose(0, 4, 2, 1, 3)
    r = np.ascontiguousarray(r).reshape(N_CORES, NF * G, NST * FD)
    return r[:, :, : NST_EMIT * FD].astype(bf16)


def _build_weights(A, signs):
    """w1 [128, 256] bf16 block coefficient mats (cluster halves);
    cm [128, 32] bf16 sign-combine mats; cv [128, 4] f32 per-partition
    constants (exp bias h0/h1, Schraudolph B2 h0/h1).

    Out partition m = kl*G + g (kl = cluster within half). Contraction
    partition p = s*G + g. Slot weights: [hi(A0), A1, A2, hi(A3), lo(A3),
    hi(A4), lo(A4), lo(A0)]; const A5 rides the activation bias."""
    bf16 = _bf16_np()

    def hi_lo(v):
        h = v.astype(bf16).astype(np.float64)
        l = (v - h).astype(bf16).astype(np.float64)
        return h, l

    a0h, a0l = hi_lo(A[:, 0])
    a3h, a3l = hi_lo(A[:, 3])
    a4h, a4l = hi_lo(A[:, 4])
    slotw = np.stack(
        [a0h, A[:, 1], A[:, 2], a3h, a3l, a4h, a4l, a0l], axis=1
    )  # [K, 8]
    const = A[:, 5]

    import ml_dtypes

    f8 = ml_dtypes.float8_e4m3fn
    logcmax = float(const.max())  # E + const <= const <= logcmax => dens <= 1
    w1 = np.zeros((NF * G, 2 * 128), np.float32)
    # cm: DoubleRow lhsT [128, 2*8, 128]: for variant v = st%8 the matmul
    # uses subtile pair (2v, 2v+1) = (half0, half1) signs, nonzero only in
    # out rows v*16..v*16+16 -- eight super-tiles accumulate into disjoint
    # 16-row stripes of one full [128, FD] PSUM tile (DoubleRow dst
    # partition base must be 0, so the out always spans all 128 rows).
    cm = np.zeros((128, 16, 128), np.float32)
    cv = np.zeros((128, 4), np.float32)
    for half in (0, 1):
        for kl in range(8):
            k = half * 8 + kl
            for g in range(G):
                m = kl * G + g
                for s in range(NF):
                    w1[s * G + g, half * 128 + m] = slotw[k, s]
                for v in range(8):
                    cm[m, v * 2 + half, v * 16 + g] = signs[k]
                cv[m, half] = const[k] - logcmax
                cv[m, 2 + half] = (const[k] - logcmax) * SCHRAUD_A1 + SCHRAUD_B1
    wp = np.zeros((128, 528), np.uint8)
    wp[:, 0:512] = w1.astype(bf16).view(np.uint8)
    wp[:, 512:528] = cv.view(np.uint8)
    return wp, cm.reshape(128, 16 * 128).astype(f8), logcmax


def _build_bass():
    nc = bacc.Bacc("TRN2", target_bir_lowering=False, debug=False)
    f32 = mybir.dt.float32
    bf16 = mybir.dt.bfloat16
    f8 = mybir.dt.float8e4
    u8 = mybir.dt.uint8
    rhs_d = nc.dram_tensor("rhs", [NF * G, NST_EMIT * FD], bf16, kind="ExternalInput")
    wp_d = nc.dram_tensor("wp", [128, 528], u8, kind="ExternalInput")
    cm_d = nc.dram_tensor("cm", [128, 16 * 128], f8, kind="ExternalInput")
    acc_d = nc.dram_tensor("acc", [128, NBLK], f32, kind="ExternalOutput")

    assert sum(RHS_CHUNKS) == NST_EMIT

    with tile.TileContext(nc) as tc:
        with (
            tc.tile_pool(name="const", bufs=1) as cpool,
            tc.tile_pool(name="rhs", bufs=RHS_BUFS) as rpool,
            tc.tile_pool(name="dens", bufs=DENS_BUFS) as dpool,
            tc.tile_pool(name="pe", bufs=E_BUFS, space="PSUM") as epool,
            tc.tile_pool(name="pg", bufs=G_BUFS, space="PSUM") as gpool,
        ):
            wp = cpool.tile([128, 528], u8)
            cm = cpool.tile([128, 16, 128], f8)
            acc = cpool.tile([128, NBLK], f32)
            scr = cpool.tile([128, FD], bf16)  # square dummy out
            wsb = cpool.tile([128, FD], bf16)  # warm-up operand
            b2m = cpool.tile([128, 2, FD], f32)  # Schraudolph bias matrix
            zro = cpool.tile([128, FD], f32)

            # rhs chunk 0 first: its transfer overlaps the remaining issue
            # chain and the program preamble.
            # Issue order = need order: packed w1+cv (first mm1/exp), rhs
            # chunk 0, remaining chunks, cm (first mm2 comes PIPE_D
            # supertiles in). Packing w1+cv into one u8 tensor saves a
            # ~640ns serialized DMA issue on the head critical path.
            nc.sync.dma_start(wp[:], wp_d[:])

            def w1h(h):
                return wp[:, h * 256 : (h + 1) * 256].bitcast(bf16)

            def cvcol(i):
                return wp[:, 512 + 4 * i : 516 + 4 * i].bitcast(f32)
            rhs_views = {}
            chunk_tiles = []
            lo = 0
            for ci, sz in enumerate(RHS_CHUNKS):
                hi = lo + sz
                rt = rpool.tile([NF * G, max(RHS_CHUNKS) * FD], bf16, tag="rhs")
                nc.sync.dma_start(rt[:, : sz * FD], rhs_d[:, lo * FD : hi * FD])
                for st in range(lo, hi):
                    rhs_views[st] = rt[:, (st - lo) * FD : (st - lo + 1) * FD]
                lo = hi
                chunk_tiles.append(rt)
                if ci == 0:
                    nc.sync.dma_start(cm[:, :, :], cm_d[:])

            # b2m[:, h, :] = cv[:, 2+h] broadcast along free: lets one DVE
            # scalar_tensor_tensor cover a whole supertile (both halves) in
            # a single op despite the per-half Schraudolph bias. Built on
            # DVE during the DMA-wait window (GpSimd elementwise measures
            # ~7.5us per [128, 512] op -- useless for compute).
            nc.vector.memset(zro[:], 0.0)
            for h in (0, 1):
                nc.vector.tensor_scalar(
                    b2m[:, h, :],
                    zro[:],
                    cvcol(2 + h),
                    None,
                    op0=mybir.AluOpType.add,
                )
            # PE warm-up on memset tiles (no DMA dependency): keeps the PE
            # busy from program start so the HAM clock-gate window fills
            # exactly while the first rhs chunk lands; real matmuls then
            # run at 2.4 GHz almost immediately.
            nc.vector.memset(wsb[:], 0.0)
            warm = gpool.tile([128, FD], f32, tag="g", name="warm")
            for _ in range(NWARM):
                nc.tensor.matmul(
                    warm[0:128, 0:WARM_FD],
                    wsb[:, 0:128],
                    wsb[:, 0:WARM_FD],
                    start=True,
                    stop=True,
                )

            dens_ring = [None] * NST
            g_ring = [None, None]

            # Greedy exp-engine load balance (ns, approximate measured
            # costs): ACT does whole-ST exp as two biased [128, FD] ops;
            # DVE does whole-ST exp as one [128, 2, FD]
            # scalar_tensor_tensor against the b2m bias matrix. Squares
            # land on ACT (one per 8-ST block).
            ACT_ST, DVE_ST, SQ = 1374.0, 1280.0, 720.0
            eng_load = {"A": 0.0, "D": 0.0}

            def emit_front(st):
                rhs = rhs_views[st]
                e = epool.tile([128, 2, FD], f32, tag="e")
                d = dpool.tile([128, 2, FD], f8, tag="dens")
                nc.tensor.matmul(
                    e[:, 0, :], w1h(0), rhs, start=True, stop=True
                )
                nc.tensor.matmul(
                    e[:, 1, :], w1h(1), rhs, start=True, stop=True
                )
                if eng_load["A"] + ACT_ST <= eng_load["D"] + DVE_ST:
                    eng_load["A"] += ACT_ST
                    for h in (0, 1):
                        nc.scalar.activation(
                            d[:, h, :],
                            e[:, h, :],
                            mybir.ActivationFunctionType.Exp,
                            bias=cvcol(h),
                        )
                else:
                    eng_load["D"] += DVE_ST
                    nc.vector.scalar_tensor_tensor(
                        d[:, :, :].bitcast(u8),
                        e[:, :, :],
                        SCHRAUD_A1,
                        b2m[:, :, :],
                        op0=mybir.AluOpType.mult,
                        op1=mybir.AluOpType.add,
                    )
                dens_ring[st] = d

            def emit_back(st):
                d = dens_ring[st]
                dens_ring[st] = None
                v = st % 8
                if v == 0:
                    g_ring[(st // 8) % 2] = gpool.tile(
                        [128, FD], f32, tag="g", name=f"g128_{st // 8}"
                    )
                g128 = g_ring[(st // 8) % 2]
                nc.tensor.matmul(
                    g128[:, :],
                    cm[:, 2 * v : 2 * v + 2, :],
                    d[:, :, :],
                    start=(v == 0),
                    stop=(v == 7 or st == NST_EMIT - 1),
                    perf_mode=mybir.MatmulPerfMode.DoubleRow,
                    skip_group_check=True,
                )
                if v == 7 or st == NST_EMIT - 1:
                    blk = st // 8
                    eng_load["A"] += SQ
                    nc.scalar.activation(
                        scr[:],
                        g128[:],
                        mybir.ActivationFunctionType.Square,
                        accum_out=acc[:, blk : blk + 1],
                    )
                    if blk == NBLK - 2:
                        nc.sync.dma_start(
                            acc_d[:, : NBLK - 1], acc[:, : NBLK - 1]
                        )

            D = PIPE_D
            for t in range(NST_EMIT + D):
                if t < NST_EMIT:
                    emit_front(t)
                if t >= D:
                    emit_back(t - D)

            nc.sync.dma_start(
                acc_d[:, NBLK - 1 : NBLK], acc[:, NBLK - 1 : NBLK]
            )

    nc.compile()
    return nc


def _get_bass():
    if "nc" not in _CACHE:
        _CACHE["nc"] = _build_bass()
    return _CACHE["nc"]


def kernel(X, means, chols, weights, it=None, **_unused):
    X = np.ascontiguousarray(np.asarray(X, np.float32))
    assert X.shape == (N_SAMPLES, 2), X.shape

    A, signs, ctr = _cluster_params(means, chols, weights)
    z = _z_term(means, chols, weights)
    wp, cm, logcmax = _build_weights(A, signs)
    rhs = _build_rhs(X, ctr)

    nc = _get_bass()
    in_maps = [
        {"rhs": rhs[c], "wp": wp, "cm": cm} for c in range(N_CORES)
    ]
    res = run_bass_kernel_spmd(nc, in_maps, core_ids=list(range(N_CORES)))

    total = 0.0
    for r in res.results:
        total += float(r["acc"].astype(np.float64).sum())
    # dens were scaled by exp(-logcmax) on device; undo on the squared sum.
    out = -(np.log(total) + 2.0 * logcmax - np.log(z)) / N_SAMPLES
    return np.float32(out)


if __name__ == "__main__":
    rng = np.random.default_rng(0)
    X = rng.standard_normal((N_SAMPLES, 2), dtype=np.float32)
    scale = 2.0 * (1.0 + rng.standard_normal((K, 1, 1), dtype=np.float32))
    chols = scale * np.ones((2, 2), np.float32)[None] + 0.5 * np.eye(2, dtype=np.float32)[None]
    means = rng.standard_normal((K, 2), dtype=np.float32)
    weights = rng.standard_normal(K, dtype=np.float32)
    print(kernel(X, means, chols, weights, 1))
